# revision 7
# baseline (speedup 1.0000x reference)
"""Trainium2 Bass kernel for top-2 MoE (nn_MoE_2113123910117).

Strategy (expert-parallel, per sharding hint):
  - Host: router logits -> softmax -> top-2 -> normalized combine weights;
    dispatch tokens to 8 expert shards (one expert per NeuronCore).
  - Device (per core): SwiGLU expert FFN over its gathered tokens,
    y = diag(scale) @ ((silu(x Wg^T) * (x Wu^T)) Wd^T), fp16 matmul
    operands with fp32 PSUM accumulation.
  - Host: scatter-add per-expert outputs back into the [B,T,D] output.

Perf structure vs the naive version:
  - wg/wu stored f-tile-major in DRAM (host pre-packs [NF*128, ND*128] so
    each f-tile is one contiguous 256KB DMA with 2KB partition lines).
  - DMA order: wg_f0, wu_f0, xt0, then remaining f-tiles -> s -> wd, so
    the first gate chain only waits for ~1.3MB instead of the whole 4MB
    wg stream. The repeat-loop back-edge barrier makes every pass pay
    this lead-in, so it directly cuts per-pass time.
  - y output in fp16 (halves output DMA; adds ~1e-4 rel err).
  - branch-prefetch hints on PE and ACT (both bodies exceed one 16KiB
    IRAM block; an unhinted back-edge stalls ~4us on the I$ fetch).

Self-contained: hardcodes all shapes from the problem spec.
"""

import os
import numpy as np

D = 1024
FF = 2048
E = 8
TOPK = 2
NCORES = 8
ND = D // 128    # 8 contraction chunks
NF = FF // 128   # 16 ff chunks
TT = 512         # token tile (moving-operand N per matmul)
MIN_CAP = 2176   # >= max expert load for the spec'd input, multiple of 128

MM_DTYPE = os.environ.get("MOE_MM_DTYPE", "float16")

LAST_RESULTS = None
_NC_CACHE = {}


def split_multi_waits(nc, mybir_mod):
    """This walrus build rejects any instruction carrying more than one
    sync wait ("Too many sync wait commands"). Hoist extra waits onto
    single-wait NOPs inserted just before the instruction on the same
    engine — semantically identical since engines execute in order."""
    n_split = 0
    for f in nc.m.functions:
        for blk in f.blocks:
            insts = blk.instructions
            newl = []
            changed = False
            for inst in insts:
                si = inst.sync_info
                if si is not None and len(si.on_wait) > 1:
                    waits = list(si.on_wait)
                    del si.on_wait[1:]
                    for j, w in enumerate(waits[1:]):
                        nop = mybir_mod.InstNoOp(
                            name=f"{inst.name}_w{j}",
                            engine=inst.engine,
                            ins=[],
                            outs=[],
                        )
                        nop.sync_info = mybir_mod.SyncInfo(on_wait=[w], on_update=[])
                        newl.append(nop)
                        n_split += 1
                    changed = True
                newl.append(inst)
            if changed:
                insts[:] = newl
    return n_split


def _token_tiles(cap):
    tiles = []
    off = 0
    while off < cap:
        tiles.append((off, min(TT, cap - off)))
        off += TT
    return tiles


def build_nc(cap, repeat=1):
    """Per-core Bass program: SwiGLU FFN for one expert over `cap` padded
    tokens. Same NEFF on all 8 cores (SPMD)."""
    import contextlib

    import concourse.bass as bass
    import concourse.mybir as mybir
    import concourse.tile as tile

    dt = mybir.dt
    f32 = dt.float32
    mmdt = getattr(dt, MM_DTYPE)
    AF = mybir.ActivationFunctionType
    NG = cap // 128  # token 128-groups

    nc = bass.Bass()
    xt = nc.dram_tensor("xt", [D, cap], mmdt, kind="ExternalInput")
    # f-tile-major packed gate/up weights: row block f is the SBUF image
    # [128, ND*128] with [p, c*128+fi] = W^T[c*128+p, f*128+fi]
    wg = nc.dram_tensor("wg", [NF * 128, ND * 128], mmdt, kind="ExternalInput")
    wu = nc.dram_tensor("wu", [NF * 128, ND * 128], mmdt, kind="ExternalInput")
    wd = nc.dram_tensor("wd", [FF, D], mmdt, kind="ExternalInput")
    sc = nc.dram_tensor("sc", [128, NG], f32, kind="ExternalInput")
    y = nc.dram_tensor("y", [cap, D], mmdt, kind="ExternalOutput")

    # Unroll two full passes per loop iteration: the intra-body pass
    # boundary has no drain/barrier, so pass i+1's lead-in DMAs (WAR on the
    # single-buffered weight tiles, which go dead ~12us before pass end)
    # overlap pass i's tail. Only every second pass pays the back-edge.
    if repeat > 1:
        assert repeat % 2 == 0, "repeat must be even (2 passes per iteration)"
        unroll = 2
        loop_iters = repeat // 2
    else:
        unroll = 1
        loop_iters = 1

    with tile.TileContext(nc) as tc:
        with (
            tc.tile_pool(name="wpool", bufs=1) as wpool,
            tc.tile_pool(name="xpool", bufs=2) as xpool,
            tc.tile_pool(name="hpool", bufs=2) as hpool,
            tc.tile_pool(name="gpool", bufs=3) as gpool,
            tc.tile_pool(name="ypool", bufs=3) as ypool,
            tc.tile_pool(name="pg", bufs=2, space="PSUM") as pgpool,
            tc.tile_pool(name="pu", bufs=2, space="PSUM") as pupool,
            tc.tile_pool(name="po", bufs=4, space="PSUM") as popool,
            (
                tc.For_i(
                    0, loop_iters, 1,
                    hint_engines=(
                        mybir.EngineType.PE,
                        mybir.EngineType.Activation,
                        mybir.EngineType.DVE,
                        mybir.EngineType.SP,
                    ),
                )
                if repeat > 1
                else contextlib.nullcontext()
            ),
        ):

            def emit_pass():
                tiles = _token_tiles(cap)
                off0, tt0 = tiles[0]
                # Lead-in critical path: the f0 gate chain needs wg_f0 + all
                # xt0 chunks, and the f0 up chain needs wu_f0 — load those
                # before the remaining 30 weight f-tiles.
                xt0 = []
                wg_sb = []
                wu_sb = []
                tg = wpool.tile([128, ND * 128], mmdt, tag="wg0")
                nc.sync.dma_start(tg[:], wg[0:128, :])
                wg_sb.append(tg)
                tu = wpool.tile([128, ND * 128], mmdt, tag="wu0")
                nc.sync.dma_start(tu[:], wu[0:128, :])
                wu_sb.append(tu)
                for d in range(ND):
                    t = xpool.tile([128, tt0], mmdt, tag=f"xt{d}")
                    nc.sync.dma_start(
                        t[:], xt[d * 128 : (d + 1) * 128, off0 : off0 + tt0]
                    )
                    xt0.append(t)
                s_sb = wpool.tile([128, NG], f32, tag="s")
                nc.sync.dma_start(s_sb[:], sc[:])
                for f in range(1, NF):
                    tg = wpool.tile([128, ND * 128], mmdt, tag=f"wg{f}")
                    nc.sync.dma_start(tg[:], wg[f * 128 : (f + 1) * 128, :])
                    wg_sb.append(tg)
                    tu = wpool.tile([128, ND * 128], mmdt, tag=f"wu{f}")
                    nc.sync.dma_start(tu[:], wu[f * 128 : (f + 1) * 128, :])
                    wu_sb.append(tu)
                wd_sb = []
                for f in range(NF):
                    t = wpool.tile([128, D], mmdt, tag=f"wd{f}")
                    nc.sync.dma_start(t[:], wd[f * 128 : (f + 1) * 128, :])
                    wd_sb.append(t)

                for off, tt in tiles:
                    if off == off0:
                        xt_t = xt0
                    else:
                        xt_t = []
                        for d in range(ND):
                            t = xpool.tile([128, tt], mmdt, tag=f"xt{d}")
                            nc.sync.dma_start(
                                t[:], xt[d * 128 : (d + 1) * 128, off : off + tt]
                            )
                            xt_t.append(t)
                    # gate/up + SwiGLU -> h^T [f, tokens]
                    ht_t = []
                    for f in range(NF):
                        pg = pgpool.tile([128, tt], f32, tag="pg")
                        pu = pupool.tile([128, tt], f32, tag="pu")
                        for d in range(ND):
                            nc.tensor.matmul(
                                pg[:],
                                wg_sb[f][:, d * 128 : (d + 1) * 128],
                                xt_t[d][:],
                                start=(d == 0),
                                stop=(d == ND - 1),
                            )
                        for d in range(ND):
                            nc.tensor.matmul(
                                pu[:],
                                wu_sb[f][:, d * 128 : (d + 1) * 128],
                                xt_t[d][:],
                                start=(d == 0),
                                stop=(d == ND - 1),
                            )
                        sg = gpool.tile([128, tt], mmdt, tag="sg")
                        nc.scalar.activation(sg[:], pg[:], AF.Silu)
                        ht = hpool.tile([128, tt], mmdt, tag=f"ht{f}")
                        nc.vector.tensor_mul(ht[:], sg[:], pu[:])
                        ht_t.append(ht)
                    # down projection, scaled by combine weight per token
                    for k in range(tt // 128):
                        g = off // 128 + k
                        po_h = []
                        for dh in range(2):
                            po = popool.tile(
                                [128, 512], f32, tag="po", name=f"po_{off}_{k}_{dh}"
                            )
                            po_h.append(po)
                        for f in range(NF):
                            lhs = ht_t[f][:, k * 128 : (k + 1) * 128]
                            for dh in range(2):
                                nc.tensor.matmul(
                                    po_h[dh][:],
                                    lhs,
                                    wd_sb[f][:, dh * 512 : (dh + 1) * 512],
                                    start=(f == 0),
                                    stop=(f == NF - 1),
                                )
                        for dh in range(2):
                            yt = ypool.tile([128, 512], mmdt, tag="yt")
                            nc.scalar.activation(
                                yt[:], po_h[dh][:], AF.Copy, scale=s_sb[:, g : g + 1]
                            )
                            nc.sync.dma_start(
                                y[off + k * 128 : off + (k + 1) * 128,
                                  dh * 512 : (dh + 1) * 512],
                                yt[:],
                            )

            for _rep in range(unroll):
                emit_pass()
    split_multi_waits(nc, mybir)
    return nc


def _pack_fmajor(WT):
    """[D, FF] -> [NF*128, ND*128] where row block f, row p, col c*128+fi =
    WT[c*128+p, f*128+fi]."""
    a = WT.reshape(ND, 128, NF, 128)      # [c, p, f, fi]
    b = a.transpose(2, 1, 0, 3)           # [f, p, c, fi]
    return np.ascontiguousarray(b.reshape(NF * 128, ND * 128))


def _get_nc(cap):
    key = (cap, MM_DTYPE)
    if key not in _NC_CACHE:
        _NC_CACHE[key] = build_nc(cap)
    return _NC_CACHE[key]


def _route(xf, Wr):
    """fp32 softmax + top-2 + normalized combine weights, matching the
    jax reference (ties broken toward lower expert index)."""
    logits = xf @ Wr.astype(np.float32).T
    m = logits.max(-1, keepdims=True)
    ex = np.exp(logits - m)
    p = ex / ex.sum(-1, keepdims=True)
    top2 = np.argsort(-p, axis=-1, kind="stable")[:, :TOPK]
    n = xf.shape[0]
    p1 = p[np.arange(n), top2[:, 0]]
    p2 = p[np.arange(n), top2[:, 1]]
    denom = (p1 + p2) + np.float32(1e-8)
    return top2, p1 / denom, p2 / denom


def make_in_maps(inputs, cap=None, mmnp=np.float16):
    x = np.asarray(inputs["x"])
    Wr = np.asarray(inputs["Wr"])
    Wg = np.asarray(inputs["Wg"])
    Wu = np.asarray(inputs["Wu"])
    Wd = np.asarray(inputs["Wd"])
    xf = x.reshape(-1, D).astype(np.float32, copy=False)
    top2, s1, s2 = _route(xf, Wr)
    xf_mm = xf.astype(mmnp)
    idxs = []
    counts = []
    for e in range(E):
        idx = np.nonzero((top2[:, 0] == e) | (top2[:, 1] == e))[0]
        idxs.append(idx)
        counts.append(len(idx))
    if cap is None:
        cap = max(MIN_CAP, -(-max(counts) // 128) * 128)
    in_maps = []
    for e in range(E):
        idx = idxs[e]
        n_e = len(idx)
        xt = np.zeros((D, cap), dtype=mmnp)
        xt[:, :n_e] = xf_mm[idx].T
        sc = np.zeros(cap, dtype=np.float32)
        sc[:n_e] = np.where(top2[idx, 0] == e, s1[idx], s2[idx])
        sc2d = np.ascontiguousarray(sc.reshape(cap // 128, 128).T)
        in_maps.append(
            {
                "xt": xt,
                "wg": _pack_fmajor(Wg[e].T.astype(np.float32)).astype(mmnp),
                "wu": _pack_fmajor(Wu[e].T.astype(np.float32)).astype(mmnp),
                "wd": np.ascontiguousarray(Wd[e].T).astype(mmnp),
                "sc": sc2d,
            }
        )
    return in_maps, idxs, cap


def kernel(**inputs):
    global LAST_RESULTS
    from concourse.bass_utils import run_bass_kernel_spmd

    x = np.asarray(inputs["x"])
    B, T, _ = x.shape
    n_tok = B * T

    mmnp = np.dtype(np.float16 if MM_DTYPE == "float16" else np.float32)
    if MM_DTYPE == "bfloat16":
        import ml_dtypes

        mmnp = np.dtype(ml_dtypes.bfloat16)

    in_maps, idxs, cap = make_in_maps(inputs, mmnp=mmnp)
    nc = _get_nc(cap)
    # The axon-tunneled devices occasionally fail a first execution with
    # NRT_EXEC_UNIT_UNRECOVERABLE; a retry on a fresh execute recovers.
    last_exc = None
    for attempt in range(3):
        try:
            res = run_bass_kernel_spmd(nc, in_maps, list(range(NCORES)))
            break
        except Exception as exc:  # noqa: BLE001
            last_exc = exc
            import time as _time

            _time.sleep(2.0)
    else:
        raise last_exc
    LAST_RESULTS = res

    out = np.zeros((n_tok, D), dtype=np.float32)
    for e in range(E):
        idx = idxs[e]
        out[idx] += res.results[e]["y"][: len(idx)]
    return out.reshape(B, T, D).astype(x.dtype, copy=False)


# revision 11
# speedup vs baseline: 1.0447x; 1.0447x over previous
"""Trainium2 Bass kernel for top-2 MoE (nn_MoE_2113123910117).

Strategy (expert-parallel, per sharding hint):
  - Host: router logits -> softmax -> top-2 -> normalized combine weights;
    dispatch tokens to 8 expert shards (one expert per NeuronCore).
  - Device (per core): SwiGLU expert FFN over its gathered tokens,
    y = diag(scale) @ ((silu(x Wg^T) * (x Wu^T)) Wd^T), fp16 matmul
    operands with fp32 PSUM accumulation.
  - Host: scatter-add per-expert outputs back into the [B,T,D] output.

Perf structure vs the naive version:
  - wg/wu stored f-tile-major in DRAM (host pre-packs [NF*128, ND*128] so
    each f-tile is one contiguous 256KB DMA with 2KB partition lines).
  - DMA order: wg_f0, wu_f0, xt0, then remaining f-tiles -> s -> wd, so
    the first gate chain only waits for ~1.3MB instead of the whole 4MB
    wg stream. The repeat-loop back-edge barrier makes every pass pay
    this lead-in, so it directly cuts per-pass time.
  - y output in fp16 (halves output DMA; adds ~1e-4 rel err).
  - branch-prefetch hints on PE and ACT (both bodies exceed one 16KiB
    IRAM block; an unhinted back-edge stalls ~4us on the I$ fetch).

Self-contained: hardcodes all shapes from the problem spec.
"""

import os
import numpy as np

D = 1024
FF = 2048
E = 8
TOPK = 2
NCORES = 8
ND = D // 128    # 8 contraction chunks
NF = FF // 128   # 16 ff chunks
TT = 512         # token tile (moving-operand N per matmul)
MIN_CAP = 2176   # >= max expert load for the spec'd input, multiple of 128

MM_DTYPE = os.environ.get("MOE_MM_DTYPE", "float16")

LAST_RESULTS = None
_NC_CACHE = {}


def split_multi_waits(nc, mybir_mod):
    """This walrus build rejects any instruction carrying more than one
    sync wait ("Too many sync wait commands"). Hoist extra waits onto
    single-wait NOPs inserted just before the instruction on the same
    engine — semantically identical since engines execute in order."""
    n_split = 0
    for f in nc.m.functions:
        for blk in f.blocks:
            insts = blk.instructions
            newl = []
            changed = False
            for inst in insts:
                si = inst.sync_info
                if si is not None and len(si.on_wait) > 1:
                    waits = list(si.on_wait)
                    del si.on_wait[1:]
                    for j, w in enumerate(waits[1:]):
                        nop = mybir_mod.InstNoOp(
                            name=f"{inst.name}_w{j}",
                            engine=inst.engine,
                            ins=[],
                            outs=[],
                        )
                        nop.sync_info = mybir_mod.SyncInfo(on_wait=[w], on_update=[])
                        newl.append(nop)
                        n_split += 1
                    changed = True
                newl.append(inst)
            if changed:
                insts[:] = newl
    return n_split


def _token_tiles(cap):
    """Token tiles covering [0, cap). The sub-512 remainder tile (if any)
    goes FIRST: its short N=128 chains need only 0.25MB of x before the
    first matmul can issue, and the remaining weight f-tiles stream in
    during its compute — shortening the per-pass lead-in stall."""
    rem = cap % TT
    tiles = []
    off = 0
    if rem:
        tiles.append((0, rem))
        off = rem
    while off < cap:
        tiles.append((off, TT))
        off += TT
    return tiles


def build_nc(cap, repeat=1, unroll=2):
    """Per-core Bass program: SwiGLU FFN for one expert over `cap` padded
    tokens. Same NEFF on all 8 cores (SPMD)."""
    import contextlib

    import concourse.bass as bass
    import concourse.mybir as mybir
    import concourse.tile as tile

    dt = mybir.dt
    f32 = dt.float32
    mmdt = getattr(dt, MM_DTYPE)
    AF = mybir.ActivationFunctionType
    NG = cap // 128  # token 128-groups

    nc = bass.Bass()
    xt = nc.dram_tensor("xt", [D, cap], mmdt, kind="ExternalInput")
    # f-tile-major packed gate/up weights: row block f is the SBUF image
    # [128, ND*128] with [p, c*128+fi] = W^T[c*128+p, f*128+fi]
    wg = nc.dram_tensor("wg", [NF * 128, ND * 128], mmdt, kind="ExternalInput")
    wu = nc.dram_tensor("wu", [NF * 128, ND * 128], mmdt, kind="ExternalInput")
    wd = nc.dram_tensor("wd", [FF, D], mmdt, kind="ExternalInput")
    sc = nc.dram_tensor("sc", [128, NG], f32, kind="ExternalInput")
    y = nc.dram_tensor("y", [cap, D], mmdt, kind="ExternalOutput")

    # Unroll `unroll` full passes per loop iteration: the intra-body pass
    # boundary has no drain/barrier, so pass i+1's lead-in DMAs (WAR on the
    # single-buffered weight tiles, which go dead ~12us before pass end)
    # overlap pass i's tail. Only every unroll-th pass pays the back-edge.
    if repeat > 1:
        assert repeat % unroll == 0, "repeat must be a multiple of unroll"
        loop_iters = repeat // unroll
    else:
        unroll = 1
        loop_iters = 1

    with tile.TileContext(nc) as tc:
        with (
            tc.tile_pool(name="wpool", bufs=1) as wpool,
            tc.tile_pool(name="xpool", bufs=3) as xpool,
            tc.tile_pool(name="hpool", bufs=2) as hpool,
            tc.tile_pool(name="gpool", bufs=3) as gpool,
            tc.tile_pool(name="ypool", bufs=3) as ypool,
            tc.tile_pool(name="pg", bufs=2, space="PSUM") as pgpool,
            tc.tile_pool(name="pu", bufs=2, space="PSUM") as pupool,
            tc.tile_pool(name="po", bufs=4, space="PSUM") as popool,
            (
                tc.For_i(
                    0, loop_iters, 1,
                    hint_engines=(
                        mybir.EngineType.PE,
                        mybir.EngineType.Activation,
                        mybir.EngineType.DVE,
                        mybir.EngineType.SP,
                    ),
                )
                if repeat > 1
                else contextlib.nullcontext()
            ),
        ):

            def emit_pass():
                tiles = _token_tiles(cap)
                off0, tt0 = tiles[0]
                # Lead-in critical path: the f0 gate chain needs wg_f0 + all
                # xt0 chunks, and the f0 up chain needs wu_f0 — load those
                # before the remaining 30 weight f-tiles.
                xt0 = []
                wg_sb = []
                wu_sb = []
                tg = wpool.tile([128, ND * 128], mmdt, tag="wg0")
                nc.sync.dma_start(tg[:], wg[0:128, :])
                wg_sb.append(tg)
                tu = wpool.tile([128, ND * 128], mmdt, tag="wu0")
                nc.sync.dma_start(tu[:], wu[0:128, :])
                wu_sb.append(tu)
                for d in range(ND):
                    t = xpool.tile([128, tt0], mmdt, tag=f"xt{d}")
                    nc.sync.dma_start(
                        t[:], xt[d * 128 : (d + 1) * 128, off0 : off0 + tt0]
                    )
                    xt0.append(t)
                s_sb = wpool.tile([128, NG], f32, tag="s")
                nc.sync.dma_start(s_sb[:], sc[:])
                for f in range(1, NF):
                    tg = wpool.tile([128, ND * 128], mmdt, tag=f"wg{f}")
                    nc.sync.dma_start(tg[:], wg[f * 128 : (f + 1) * 128, :])
                    wg_sb.append(tg)
                    tu = wpool.tile([128, ND * 128], mmdt, tag=f"wu{f}")
                    nc.sync.dma_start(tu[:], wu[f * 128 : (f + 1) * 128, :])
                    wu_sb.append(tu)
                wd_sb = []
                for f in range(NF):
                    t = wpool.tile([128, D], mmdt, tag=f"wd{f}")
                    nc.sync.dma_start(t[:], wd[f * 128 : (f + 1) * 128, :])
                    wd_sb.append(t)

                for off, tt in tiles:
                    if off == off0:
                        xt_t = xt0
                    else:
                        xt_t = []
                        for d in range(ND):
                            t = xpool.tile([128, tt], mmdt, tag=f"xt{d}")
                            nc.sync.dma_start(
                                t[:], xt[d * 128 : (d + 1) * 128, off : off + tt]
                            )
                            xt_t.append(t)
                    # gate/up + SwiGLU -> h^T [f, tokens]
                    ht_t = []
                    for f in range(NF):
                        pg = pgpool.tile([128, tt], f32, tag="pg")
                        pu = pupool.tile([128, tt], f32, tag="pu")
                        for d in range(ND):
                            nc.tensor.matmul(
                                pg[:],
                                wg_sb[f][:, d * 128 : (d + 1) * 128],
                                xt_t[d][:],
                                start=(d == 0),
                                stop=(d == ND - 1),
                            )
                        for d in range(ND):
                            nc.tensor.matmul(
                                pu[:],
                                wu_sb[f][:, d * 128 : (d + 1) * 128],
                                xt_t[d][:],
                                start=(d == 0),
                                stop=(d == ND - 1),
                            )
                        sg = gpool.tile([128, tt], mmdt, tag="sg")
                        nc.scalar.activation(sg[:], pg[:], AF.Silu)
                        ht = hpool.tile([128, tt], mmdt, tag=f"ht{f}")
                        nc.vector.tensor_mul(ht[:], sg[:], pu[:])
                        ht_t.append(ht)
                    # down projection, scaled by combine weight per token
                    for k in range(tt // 128):
                        g = off // 128 + k
                        po_h = []
                        for dh in range(2):
                            po = popool.tile(
                                [128, 512], f32, tag="po", name=f"po_{off}_{k}_{dh}"
                            )
                            po_h.append(po)
                        for f in range(NF):
                            lhs = ht_t[f][:, k * 128 : (k + 1) * 128]
                            for dh in range(2):
                                nc.tensor.matmul(
                                    po_h[dh][:],
                                    lhs,
                                    wd_sb[f][:, dh * 512 : (dh + 1) * 512],
                                    start=(f == 0),
                                    stop=(f == NF - 1),
                                )
                        for dh in range(2):
                            yt = ypool.tile([128, 512], mmdt, tag="yt")
                            nc.scalar.activation(
                                yt[:], po_h[dh][:], AF.Copy, scale=s_sb[:, g : g + 1]
                            )
                            nc.sync.dma_start(
                                y[off + k * 128 : off + (k + 1) * 128,
                                  dh * 512 : (dh + 1) * 512],
                                yt[:],
                            )

            for _rep in range(unroll):
                emit_pass()
    split_multi_waits(nc, mybir)
    return nc


def _pack_fmajor(WT):
    """[D, FF] -> [NF*128, ND*128] where row block f, row p, col c*128+fi =
    WT[c*128+p, f*128+fi]."""
    a = WT.reshape(ND, 128, NF, 128)      # [c, p, f, fi]
    b = a.transpose(2, 1, 0, 3)           # [f, p, c, fi]
    return np.ascontiguousarray(b.reshape(NF * 128, ND * 128))


def _get_nc(cap):
    key = (cap, MM_DTYPE)
    if key not in _NC_CACHE:
        _NC_CACHE[key] = build_nc(cap)
    return _NC_CACHE[key]


def _route(xf, Wr):
    """fp32 softmax + top-2 + normalized combine weights, matching the
    jax reference (ties broken toward lower expert index)."""
    logits = xf @ Wr.astype(np.float32).T
    m = logits.max(-1, keepdims=True)
    ex = np.exp(logits - m)
    p = ex / ex.sum(-1, keepdims=True)
    top2 = np.argsort(-p, axis=-1, kind="stable")[:, :TOPK]
    n = xf.shape[0]
    p1 = p[np.arange(n), top2[:, 0]]
    p2 = p[np.arange(n), top2[:, 1]]
    denom = (p1 + p2) + np.float32(1e-8)
    return top2, p1 / denom, p2 / denom


def make_in_maps(inputs, cap=None, mmnp=np.float16):
    x = np.asarray(inputs["x"])
    Wr = np.asarray(inputs["Wr"])
    Wg = np.asarray(inputs["Wg"])
    Wu = np.asarray(inputs["Wu"])
    Wd = np.asarray(inputs["Wd"])
    xf = x.reshape(-1, D).astype(np.float32, copy=False)
    top2, s1, s2 = _route(xf, Wr)
    xf_mm = xf.astype(mmnp)
    idxs = []
    counts = []
    for e in range(E):
        idx = np.nonzero((top2[:, 0] == e) | (top2[:, 1] == e))[0]
        idxs.append(idx)
        counts.append(len(idx))
    if cap is None:
        cap = max(MIN_CAP, -(-max(counts) // 128) * 128)
    in_maps = []
    for e in range(E):
        idx = idxs[e]
        n_e = len(idx)
        xt = np.zeros((D, cap), dtype=mmnp)
        xt[:, :n_e] = xf_mm[idx].T
        sc = np.zeros(cap, dtype=np.float32)
        sc[:n_e] = np.where(top2[idx, 0] == e, s1[idx], s2[idx])
        sc2d = np.ascontiguousarray(sc.reshape(cap // 128, 128).T)
        in_maps.append(
            {
                "xt": xt,
                "wg": _pack_fmajor(Wg[e].T.astype(np.float32)).astype(mmnp),
                "wu": _pack_fmajor(Wu[e].T.astype(np.float32)).astype(mmnp),
                "wd": np.ascontiguousarray(Wd[e].T).astype(mmnp),
                "sc": sc2d,
            }
        )
    return in_maps, idxs, cap


def kernel(**inputs):
    global LAST_RESULTS
    from concourse.bass_utils import run_bass_kernel_spmd

    x = np.asarray(inputs["x"])
    B, T, _ = x.shape
    n_tok = B * T

    mmnp = np.dtype(np.float16 if MM_DTYPE == "float16" else np.float32)
    if MM_DTYPE == "bfloat16":
        import ml_dtypes

        mmnp = np.dtype(ml_dtypes.bfloat16)

    in_maps, idxs, cap = make_in_maps(inputs, mmnp=mmnp)
    nc = _get_nc(cap)
    # The axon-tunneled devices occasionally fail a first execution with
    # NRT_EXEC_UNIT_UNRECOVERABLE; a retry on a fresh execute recovers.
    last_exc = None
    for attempt in range(3):
        try:
            res = run_bass_kernel_spmd(nc, in_maps, list(range(NCORES)))
            break
        except Exception as exc:  # noqa: BLE001
            last_exc = exc
            import time as _time

            _time.sleep(2.0)
    else:
        raise last_exc
    LAST_RESULTS = res

    out = np.zeros((n_tok, D), dtype=np.float32)
    for e in range(E):
        idx = idxs[e]
        out[idx] += res.results[e]["y"][: len(idx)]
    return out.reshape(B, T, D).astype(x.dtype, copy=False)


# revision 13
# speedup vs baseline: 1.2014x; 1.1501x over previous
"""Trainium2 Bass kernel for top-2 MoE (nn_MoE_2113123910117).

Strategy (expert-parallel, per sharding hint):
  - Host: router logits -> softmax -> top-2 -> normalized combine weights;
    dispatch tokens to 8 expert shards (one expert per NeuronCore).
  - Device (per core): SwiGLU expert FFN over its gathered tokens,
    y = diag(scale) @ ((silu(x Wg^T) * (x Wu^T)) Wd^T), fp16 matmul
    operands with fp32 PSUM accumulation.
  - Host: scatter-add per-expert outputs back into the [B,T,D] output.

Perf structure vs the naive version:
  - wg/wu stored f-tile-major in DRAM (host pre-packs [NF*128, ND*128] so
    each f-tile is one contiguous 256KB DMA with 2KB partition lines).
  - DMA order: wg_f0, wu_f0, xt0, then remaining f-tiles -> s -> wd, so
    the first gate chain only waits for ~1.3MB instead of the whole 4MB
    wg stream. The repeat-loop back-edge barrier makes every pass pay
    this lead-in, so it directly cuts per-pass time.
  - y output in fp16 (halves output DMA; adds ~1e-4 rel err).
  - branch-prefetch hints on PE and ACT (both bodies exceed one 16KiB
    IRAM block; an unhinted back-edge stalls ~4us on the I$ fetch).

Self-contained: hardcodes all shapes from the problem spec.
"""

import os
import numpy as np

D = 1024
FF = 2048
E = 8
TOPK = 2
NCORES = 8
ND = D // 128    # 8 contraction chunks
NF = FF // 128   # 16 ff chunks
TT = 512         # token tile (moving-operand N per matmul)
MIN_CAP = 2176   # >= max expert load for the spec'd input, multiple of 128

MM_DTYPE = os.environ.get("MOE_MM_DTYPE", "float16")

LAST_RESULTS = None
_NC_CACHE = {}


def split_multi_waits(nc, mybir_mod):
    """This walrus build rejects any instruction carrying more than one
    sync wait ("Too many sync wait commands"). Hoist extra waits onto
    single-wait NOPs inserted just before the instruction on the same
    engine — semantically identical since engines execute in order."""
    n_split = 0
    for f in nc.m.functions:
        for blk in f.blocks:
            insts = blk.instructions
            newl = []
            changed = False
            for inst in insts:
                si = inst.sync_info
                if si is not None and len(si.on_wait) > 1:
                    waits = list(si.on_wait)
                    del si.on_wait[1:]
                    for j, w in enumerate(waits[1:]):
                        nop = mybir_mod.InstNoOp(
                            name=f"{inst.name}_w{j}",
                            engine=inst.engine,
                            ins=[],
                            outs=[],
                        )
                        nop.sync_info = mybir_mod.SyncInfo(on_wait=[w], on_update=[])
                        newl.append(nop)
                        n_split += 1
                    changed = True
                newl.append(inst)
            if changed:
                insts[:] = newl
    return n_split


def _token_tiles(cap):
    """Token tiles covering [0, cap), full 512-token tiles first. The first
    tile must be a FULL tile: its ~80us of compute is what covers the 13MB
    weight/x DMA stream; leading with the short remainder tile shortens the
    lead-in by ~2us but exposes a ~17us weight-stream stall right after
    (measured in TimelineSim: 377us vs 360us)."""
    tiles = []
    off = 0
    while off < cap:
        tiles.append((off, min(TT, cap - off)))
        off += TT
    return tiles


def build_nc(cap, repeat=1, unroll=2):
    """Per-core Bass program: SwiGLU FFN for one expert over `cap` padded
    tokens. Same NEFF on all 8 cores (SPMD)."""
    import contextlib

    import concourse.bass as bass
    import concourse.mybir as mybir
    import concourse.tile as tile

    dt = mybir.dt
    f32 = dt.float32
    mmdt = getattr(dt, MM_DTYPE)
    AF = mybir.ActivationFunctionType
    NG = cap // 128  # token 128-groups

    nc = bass.Bass()
    xt = nc.dram_tensor("xt", [D, cap], mmdt, kind="ExternalInput")
    # f-tile-major packed gate/up weights: row block f is the SBUF image
    # [128, ND*128] with [p, c*128+fi] = W^T[c*128+p, f*128+fi]
    wg = nc.dram_tensor("wg", [NF * 128, ND * 128], mmdt, kind="ExternalInput")
    wu = nc.dram_tensor("wu", [NF * 128, ND * 128], mmdt, kind="ExternalInput")
    wd = nc.dram_tensor("wd", [FF, D], mmdt, kind="ExternalInput")
    sc = nc.dram_tensor("sc", [128, NG], f32, kind="ExternalInput")
    y = nc.dram_tensor("y", [cap, D], mmdt, kind="ExternalOutput")

    # Unroll `unroll` full passes per loop iteration: the intra-body pass
    # boundary has no drain/barrier, so pass i+1's lead-in DMAs (WAR on the
    # single-buffered weight tiles, which go dead ~12us before pass end)
    # overlap pass i's tail. Only every unroll-th pass pays the back-edge.
    if repeat > 1:
        assert repeat % unroll == 0, "repeat must be a multiple of unroll"
        loop_iters = repeat // unroll
    else:
        unroll = 1
        loop_iters = 1

    with tile.TileContext(nc) as tc:
        with (
            tc.tile_pool(name="wpool", bufs=1) as wpool,
            tc.tile_pool(name="xpool", bufs=3) as xpool,
            tc.tile_pool(name="hpool", bufs=2) as hpool,
            tc.tile_pool(name="gpool", bufs=3) as gpool,
            tc.tile_pool(name="ypool", bufs=3) as ypool,
            tc.tile_pool(name="pg", bufs=2, space="PSUM") as pgpool,
            tc.tile_pool(name="pu", bufs=2, space="PSUM") as pupool,
            tc.tile_pool(name="po", bufs=4, space="PSUM") as popool,
            (
                tc.For_i(
                    0, loop_iters, 1,
                    hint_engines=(
                        mybir.EngineType.PE,
                        mybir.EngineType.Activation,
                        mybir.EngineType.DVE,
                        mybir.EngineType.SP,
                    ),
                )
                if repeat > 1
                else contextlib.nullcontext()
            ),
        ):

            def emit_pass():
                tiles = _token_tiles(cap)
                off0, tt0 = tiles[0]
                # Lead-in critical path: the f0 gate chain needs wg_f0 + all
                # xt0 chunks, and the f0 up chain needs wu_f0 — load those
                # before the remaining 30 weight f-tiles.
                xt0 = []
                wg_sb = []
                wu_sb = []
                tg = wpool.tile([128, ND * 128], mmdt, tag="wg0")
                nc.sync.dma_start(tg[:], wg[0:128, :])
                wg_sb.append(tg)
                tu = wpool.tile([128, ND * 128], mmdt, tag="wu0")
                nc.sync.dma_start(tu[:], wu[0:128, :])
                wu_sb.append(tu)
                for d in range(ND):
                    t = xpool.tile([128, tt0], mmdt, tag=f"xt{d}")
                    nc.sync.dma_start(
                        t[:], xt[d * 128 : (d + 1) * 128, off0 : off0 + tt0]
                    )
                    xt0.append(t)
                s_sb = wpool.tile([128, NG], f32, tag="s")
                nc.sync.dma_start(s_sb[:], sc[:])
                for f in range(1, NF):
                    tg = wpool.tile([128, ND * 128], mmdt, tag=f"wg{f}")
                    nc.sync.dma_start(tg[:], wg[f * 128 : (f + 1) * 128, :])
                    wg_sb.append(tg)
                    tu = wpool.tile([128, ND * 128], mmdt, tag=f"wu{f}")
                    nc.sync.dma_start(tu[:], wu[f * 128 : (f + 1) * 128, :])
                    wu_sb.append(tu)
                wd_sb = []
                for f in range(NF):
                    t = wpool.tile([128, D], mmdt, tag=f"wd{f}")
                    nc.sync.dma_start(t[:], wd[f * 128 : (f + 1) * 128, :])
                    wd_sb.append(t)

                for off, tt in tiles:
                    if off == off0:
                        xt_t = xt0
                    else:
                        xt_t = []
                        for d in range(ND):
                            t = xpool.tile([128, tt], mmdt, tag=f"xt{d}")
                            nc.sync.dma_start(
                                t[:], xt[d * 128 : (d + 1) * 128, off : off + tt]
                            )
                            xt_t.append(t)
                    # gate/up + SwiGLU -> h^T [f, tokens]
                    ht_t = []
                    for f in range(NF):
                        pg = pgpool.tile([128, tt], f32, tag="pg")
                        pu = pupool.tile([128, tt], f32, tag="pu")
                        for d in range(ND):
                            nc.tensor.matmul(
                                pg[:],
                                wg_sb[f][:, d * 128 : (d + 1) * 128],
                                xt_t[d][:],
                                start=(d == 0),
                                stop=(d == ND - 1),
                            )
                        for d in range(ND):
                            nc.tensor.matmul(
                                pu[:],
                                wu_sb[f][:, d * 128 : (d + 1) * 128],
                                xt_t[d][:],
                                start=(d == 0),
                                stop=(d == ND - 1),
                            )
                        sg = gpool.tile([128, tt], mmdt, tag="sg")
                        nc.scalar.activation(sg[:], pg[:], AF.Silu)
                        ht = hpool.tile([128, tt], mmdt, tag=f"ht{f}")
                        nc.vector.tensor_mul(ht[:], sg[:], pu[:])
                        ht_t.append(ht)
                    # down projection, scaled by combine weight per token
                    for k in range(tt // 128):
                        g = off // 128 + k
                        po_h = []
                        for dh in range(2):
                            po = popool.tile(
                                [128, 512], f32, tag="po", name=f"po_{off}_{k}_{dh}"
                            )
                            po_h.append(po)
                        # For the final group of the pass, run the two
                        # D-half chains dh-major (all 16 f of dh=0, then
                        # dh=1): the dh=0 chain then finishes ~3.4us early
                        # and its ACT+DMA drain overlaps the dh=1 chain,
                        # shortening the post-last-matmul tail. Elsewhere
                        # keep f-major (one LDWEIGHTS serves both halves).
                        final_group = g == NG - 1
                        if final_group:
                            for dh in range(2):
                                for f in range(NF):
                                    lhs = ht_t[f][:, k * 128 : (k + 1) * 128]
                                    nc.tensor.matmul(
                                        po_h[dh][:],
                                        lhs,
                                        wd_sb[f][:, dh * 512 : (dh + 1) * 512],
                                        start=(f == 0),
                                        stop=(f == NF - 1),
                                    )
                                yt = ypool.tile([128, 512], mmdt, tag="yt")
                                nc.scalar.activation(
                                    yt[:], po_h[dh][:], AF.Copy,
                                    scale=s_sb[:, g : g + 1],
                                )
                                nc.sync.dma_start(
                                    y[off + k * 128 : off + (k + 1) * 128,
                                      dh * 512 : (dh + 1) * 512],
                                    yt[:],
                                )
                        else:
                            for f in range(NF):
                                lhs = ht_t[f][:, k * 128 : (k + 1) * 128]
                                for dh in range(2):
                                    nc.tensor.matmul(
                                        po_h[dh][:],
                                        lhs,
                                        wd_sb[f][:, dh * 512 : (dh + 1) * 512],
                                        start=(f == 0),
                                        stop=(f == NF - 1),
                                    )
                            for dh in range(2):
                                yt = ypool.tile([128, 512], mmdt, tag="yt")
                                nc.scalar.activation(
                                    yt[:], po_h[dh][:], AF.Copy,
                                    scale=s_sb[:, g : g + 1],
                                )
                                nc.sync.dma_start(
                                    y[off + k * 128 : off + (k + 1) * 128,
                                      dh * 512 : (dh + 1) * 512],
                                    yt[:],
                                )

            for _rep in range(unroll):
                emit_pass()
    split_multi_waits(nc, mybir)
    return nc


def _pack_fmajor(WT):
    """[D, FF] -> [NF*128, ND*128] where row block f, row p, col c*128+fi =
    WT[c*128+p, f*128+fi]."""
    a = WT.reshape(ND, 128, NF, 128)      # [c, p, f, fi]
    b = a.transpose(2, 1, 0, 3)           # [f, p, c, fi]
    return np.ascontiguousarray(b.reshape(NF * 128, ND * 128))


def _get_nc(cap):
    key = (cap, MM_DTYPE)
    if key not in _NC_CACHE:
        _NC_CACHE[key] = build_nc(cap)
    return _NC_CACHE[key]


def _route(xf, Wr):
    """fp32 softmax + top-2 + normalized combine weights, matching the
    jax reference (ties broken toward lower expert index)."""
    logits = xf @ Wr.astype(np.float32).T
    m = logits.max(-1, keepdims=True)
    ex = np.exp(logits - m)
    p = ex / ex.sum(-1, keepdims=True)
    top2 = np.argsort(-p, axis=-1, kind="stable")[:, :TOPK]
    n = xf.shape[0]
    p1 = p[np.arange(n), top2[:, 0]]
    p2 = p[np.arange(n), top2[:, 1]]
    denom = (p1 + p2) + np.float32(1e-8)
    return top2, p1 / denom, p2 / denom


def make_in_maps(inputs, cap=None, mmnp=np.float16):
    x = np.asarray(inputs["x"])
    Wr = np.asarray(inputs["Wr"])
    Wg = np.asarray(inputs["Wg"])
    Wu = np.asarray(inputs["Wu"])
    Wd = np.asarray(inputs["Wd"])
    xf = x.reshape(-1, D).astype(np.float32, copy=False)
    top2, s1, s2 = _route(xf, Wr)
    xf_mm = xf.astype(mmnp)
    idxs = []
    counts = []
    for e in range(E):
        idx = np.nonzero((top2[:, 0] == e) | (top2[:, 1] == e))[0]
        idxs.append(idx)
        counts.append(len(idx))
    if cap is None:
        cap = max(MIN_CAP, -(-max(counts) // 128) * 128)
    in_maps = []
    for e in range(E):
        idx = idxs[e]
        n_e = len(idx)
        xt = np.zeros((D, cap), dtype=mmnp)
        xt[:, :n_e] = xf_mm[idx].T
        sc = np.zeros(cap, dtype=np.float32)
        sc[:n_e] = np.where(top2[idx, 0] == e, s1[idx], s2[idx])
        sc2d = np.ascontiguousarray(sc.reshape(cap // 128, 128).T)
        in_maps.append(
            {
                "xt": xt,
                "wg": _pack_fmajor(Wg[e].T.astype(np.float32)).astype(mmnp),
                "wu": _pack_fmajor(Wu[e].T.astype(np.float32)).astype(mmnp),
                "wd": np.ascontiguousarray(Wd[e].T).astype(mmnp),
                "sc": sc2d,
            }
        )
    return in_maps, idxs, cap


def kernel(**inputs):
    global LAST_RESULTS
    from concourse.bass_utils import run_bass_kernel_spmd

    x = np.asarray(inputs["x"])
    B, T, _ = x.shape
    n_tok = B * T

    mmnp = np.dtype(np.float16 if MM_DTYPE == "float16" else np.float32)
    if MM_DTYPE == "bfloat16":
        import ml_dtypes

        mmnp = np.dtype(ml_dtypes.bfloat16)

    in_maps, idxs, cap = make_in_maps(inputs, mmnp=mmnp)
    nc = _get_nc(cap)
    # The axon-tunneled devices occasionally fail a first execution with
    # NRT_EXEC_UNIT_UNRECOVERABLE; a retry on a fresh execute recovers.
    last_exc = None
    for attempt in range(3):
        try:
            res = run_bass_kernel_spmd(nc, in_maps, list(range(NCORES)))
            break
        except Exception as exc:  # noqa: BLE001
            last_exc = exc
            import time as _time

            _time.sleep(2.0)
    else:
        raise last_exc
    LAST_RESULTS = res

    out = np.zeros((n_tok, D), dtype=np.float32)
    for e in range(E):
        idx = idxs[e]
        out[idx] += res.results[e]["y"][: len(idx)]
    return out.reshape(B, T, D).astype(x.dtype, copy=False)
